# revision 15
# baseline (speedup 1.0000x reference)
"""Trainium2 Bass kernel for the DeepFace-style CNN (nn_DeepFace_10574209482846).

Sharding: pure data parallel - batch 2048 split as 256 images per core
across 8 cores; all weights replicated (host-preprocessed into matmul-
friendly layouts, cast to bf16).

v3: software-pipelined phase 1 (conv1 of sb+1 ahead of conv2a of sb,
so the PE never drains on conv1's activations and the HAM clock stays
warm) + conv2b tanh into contiguous staging with the (y,x,b) scatter
moved to the Vector engine (writes base + replica rows directly).

v2: tap-packed K layouts to cut PE stream cycles:
 - conv1: (grp, ci, di) packed on K (K=60), 3 accumulating matmuls (dj)
   over di-shifted x replicas loaded straight from DRAM.
 - conv2a: 2-group pairs with a +1-column replica on partitions 64..127
   (K=128): 3 pair-matmuls (dj=0,1) + 3 singles (dj=2, K=64).
 - conv2b: 9 taps, K=128 2-group block-diagonal (unchanged).
 - locally-connected stack: batch-contiguous (y, x, b) layout with
   +1-position replica rows, tap-pair matmuls (unchanged).
All inter-layer pools double-buffered so sub-batches pipeline with no
PE gaps (keeps the PE HAM clock at 2.4 GHz).
"""

import numpy as np
import concourse.bass as bass
import concourse.bacc as bacc
import concourse.tile as tile
import concourse.mybir as mybir
from concourse import bass_utils

bf16 = mybir.dt.bfloat16
f32 = mybir.dt.float32
BF = mybir.dt.np(bf16)

N_CORES = 8
B_FULL = 2048
B_CORE = 256          # images per core
SB = 8                # images per sub-batch (2 per group)
NSB = B_CORE // SB    # 32
BSB = SB // 4         # 2 images per group per sub-batch

L1 = BSB * 841        # h1 elements per pair tile (flat (b, 29, 29))

_CACHE = {}


def _build_module(nsb=NSB):
    nc = bacc.Bacc("TRN2", target_bir_lowering=False, debug=False,
                   enable_asserts=True, num_devices=N_CORES)

    # ---- DRAM I/O ----
    x_d = nc.dram_tensor("x", [B_CORE, 5, 3600], bf16, kind="ExternalInput").ap()
    w1di_d = nc.dram_tensor("w1di", [60, 3 * 128], bf16, kind="ExternalInput").ap()
    b1t_d = nc.dram_tensor("b1t", [128, 1], f32, kind="ExternalInput").ap()
    w2ap_d = nc.dram_tensor("w2ap", [128, 3 * 128], bf16, kind="ExternalInput").ap()
    w2as_d = nc.dram_tensor("w2as", [64, 3 * 128], bf16, kind="ExternalInput").ap()
    b2at_d = nc.dram_tensor("b2at", [128, 1], f32, kind="ExternalInput").ap()
    w2bbd_d = nc.dram_tensor("w2bbd", [128, 9 * 128], bf16, kind="ExternalInput").ap()
    b2bt_d = nc.dram_tensor("b2bt", [128, 1], f32, kind="ExternalInput").ap()
    lw3n_d = nc.dram_tensor("lw3n", [45, 128, 1920], bf16, kind="ExternalInput").ap()
    lb3_d = nc.dram_tensor("lb3t", [64, 81], f32, kind="ExternalInput").ap()
    lw4n_d = nc.dram_tensor("lw4n", [15, 128, 1920], bf16, kind="ExternalInput").ap()
    lb4_d = nc.dram_tensor("lb4t", [64, 25], f32, kind="ExternalInput").ap()
    lw5n_d = nc.dram_tensor("lw5n", [6, 128, 768], bf16, kind="ExternalInput").ap()
    lb5_d = nc.dram_tensor("lb5t", [64, 9], f32, kind="ExternalInput").ap()
    hwch_d = nc.dram_tensor("hwch", [64, 18], bf16, kind="ExternalInput").ap()
    logits_d = nc.dram_tensor("logits", [2, B_CORE], f32, kind="ExternalOutput").ap()

    Tanh = mybir.ActivationFunctionType.Tanh

    with tile.TileContext(nc) as tc:
        with (
            tc.tile_pool(name="wp", bufs=1) as wp,
            tc.tile_pool(name="lwp", bufs=4) as lwp,
            tc.tile_pool(name="xp", bufs=2) as xp,
            tc.tile_pool(name="h1p", bufs=2) as h1p,
            tc.tile_pool(name="h2ap", bufs=2) as h2ap,
            tc.tile_pool(name="stp", bufs=2) as stp,
            tc.tile_pool(name="big1", bufs=1) as big1,
            tc.tile_pool(name="big2", bufs=1) as big2,
            tc.tile_pool(name="cps", bufs=5, space="PSUM") as cps,
            tc.tile_pool(name="lps", bufs=2, space="PSUM") as lps,
            tc.tile_pool(name="hps", bufs=1, space="PSUM") as hps,
        ):
            # ---- persistent weights ----
            w1di = wp.tile([60, 3 * 128], bf16)
            nc.sync.dma_start(w1di[:], w1di_d[:])
            b1t = wp.tile([128, 1], f32)
            nc.sync.dma_start(b1t[:], b1t_d[:])
            w2ap = wp.tile([128, 3 * 128], bf16)
            nc.sync.dma_start(w2ap[:], w2ap_d[:])
            w2as = wp.tile([64, 3 * 128], bf16)
            nc.sync.dma_start(w2as[:], w2as_d[:])
            b2at = wp.tile([128, 1], f32)
            nc.sync.dma_start(b2at[:], b2at_d[:])
            w2bbd = wp.tile([128, 9 * 128], bf16)
            nc.sync.dma_start(w2bbd[:], w2bbd_d[:])
            b2bt = wp.tile([128, 1], f32)
            nc.sync.dma_start(b2bt[:], b2bt_d[:])
            lb3t = wp.tile([64, 81], f32)
            nc.sync.dma_start(lb3t[:], lb3_d[:])
            lb4t = wp.tile([64, 25], f32)
            nc.sync.dma_start(lb4t[:], lb4_d[:])
            lb5t = wp.tile([64, 9], f32)
            nc.sync.dma_start(lb5t[:], lb5_d[:])
            hwch = wp.tile([64, 18], bf16)
            nc.sync.dma_start(hwch[:], hwch_d[:])

            # ---- persistent activations, batch-contiguous (y, x, b) with a
            # 256-col leading pad; rows 64-127 hold the +1-position replica
            # (element e lives at col PAD+e on rows 0-63 and col e on 64-127).
            # h4rep/h5t reuse the big slots once h2brep/h3rep are dead.
            h2brep = big1.tile([128, 256 + 169 * 256], bf16, tag="b1")
            # zero the one-past-the-end replica column block (read with zero
            # weights by the edge position-pair matmuls; must not be NaN)
            nc.gpsimd.memset(h2brep[64:128, 169 * 256:170 * 256], 0.0)

            # (c, b, p) views of base and replica halves, p = y*13 + x
            h2b_bv = h2brep[0:64, 256:].rearrange("c (p b) -> c b p",
                                                  p=169, b=256)
            h2b_rv = h2brep[64:128, 0:169 * 256].rearrange("c (p b) -> c b p",
                                                           p=169, b=256)

            # ======== phase 1: conv1 -> conv2a -> conv2b, sw-pipelined =====
            # conv1 of sub-batch sb+1 issues before conv2a of sb so the PE
            # never waits on conv1's activation drain (and the HAM clock
            # stays warm); conv2b tanh lands in a contiguous staging tile,
            # the (y,x,b) scatter runs on the idle Vector engine (both base
            # and +1-position replica rows, so no bulk replica copy later).
            def x_dma(sb):
                # x tile: rows 20*di + 5*g + ci hold x[ci] shifted di rows up
                x3 = xp.tile([60, BSB * 3600], bf16, tag="x")
                for di in range(3):
                    for g in range(4):
                        b0 = 64 * g + BSB * sb
                        src = x_d[b0:b0 + BSB, :, 60 * di:].rearrange(
                            "b c m -> c b m")
                        dst = x3[20 * di + 5 * g:20 * di + 5 * g + 5,
                                 :].rearrange(
                            "c (b m) -> c b m", b=BSB)[:, :, :3600 - 60 * di]
                        nc.sync.dma_start(dst, src)
                return x3

            def conv1(x3):
                # K=60 (4 groups x 5ci x 3di), 3 dj-matmuls
                xv = x3[:].rearrange("c (b h w) -> c b h w", b=BSB, h=60, w=60)
                h1pr = {}
                for r in range(2):
                    h1pr[r] = h1p.tile([128, 1 + L1 + 3], bf16, tag=f"h1{r}",
                                       name=f"h1pair{r}")
                for (y0, ny) in [(0, 8), (8, 8), (16, 8), (24, 5)]:
                    ps = cps.tile([128, BSB * 8 * 29], f32, tag="cps")
                    psw = ps[:, :BSB * ny * 29]
                    for dj in range(3):
                        rhs = xv[:, :, 2 * y0: 2 * y0 + 2 * ny - 1: 2,
                                 dj: dj + 57: 2]
                        nc.tensor.matmul(psw, w1di[:, 128 * dj:128 * (dj + 1)],
                                         rhs, start=(dj == 0), stop=(dj == 2))
                    psv = psw.rearrange("c (b y x) -> c b y x",
                                        b=BSB, y=ny, x=29)
                    for r in range(2):
                        dstv = h1pr[r][0:64, 1:1 + L1].rearrange(
                            "c (b h w) -> c b h w", b=BSB, h=29, w=29)
                        nc.scalar.activation(dstv[:, :, y0:y0 + ny, :],
                                             psv[64 * r:64 * (r + 1)],
                                             Tanh, bias=b1t[0:64])
                # +1-element replica rows (gives dj+1 taps in pair matmuls)
                for r in range(2):
                    nc.vector.tensor_copy(h1pr[r][64:128, 0:L1],
                                          h1pr[r][0:64, 1:1 + L1])
                return h1pr

            def conv2a(h1pr):
                # per pair, 3 pair-matmuls (K=128) + 3 singles
                h2a_t = {}
                for r in range(2):
                    h2a_t[r] = h2ap.tile([128, BSB * 729], bf16, tag=f"h2a{r}",
                                         name=f"h2a{r}")
                    h2av = h2a_t[r][:].rearrange("c (b h w) -> c b h w",
                                                 b=BSB, h=27, w=27)
                    basev = h1pr[r][:, 1:1 + L1].rearrange(
                        "c (b h w) -> c b h w", b=BSB, h=29, w=29)
                    sglv = h1pr[r][0:64, 3:3 + L1].rearrange(
                        "c (b h w) -> c b h w", b=BSB, h=29, w=29)
                    for (y0, ny) in [(0, 9), (9, 9), (18, 9)]:
                        ps = cps.tile([128, BSB * 9 * 27], f32, tag="cps")
                        psw = ps[:, :BSB * ny * 27]
                        for di in range(3):
                            rhs = basev[:, :, y0 + di: y0 + di + ny, 0:27]
                            nc.tensor.matmul(
                                psw, w2ap[:, 128 * di:128 * (di + 1)], rhs,
                                start=(di == 0), stop=False)
                        for di in range(3):
                            rhs = sglv[:, :, y0 + di: y0 + di + ny, 0:27]
                            nc.tensor.matmul(
                                psw, w2as[:, 128 * di:128 * (di + 1)], rhs,
                                start=False, stop=(di == 2))
                        nc.scalar.activation(h2av[:, :, y0:y0 + ny, :], psw,
                                             Tanh, bias=b2at[:])
                return h2a_t

            TAPS3 = [(di, dj) for di in range(3) for dj in range(3)]

            def conv2b(sb, h2a_t):
                # per pair, K=128 block-diag, stride 2
                for r in range(2):
                    h2av = h2a_t[r][:].rearrange("c (b h w) -> c b h w",
                                                 b=BSB, h=27, w=27)
                    ps = cps.tile([128, BSB * 169], f32, tag="cps")
                    for t, (di, dj) in enumerate(TAPS3):
                        rhs = h2av[:, :, di: di + 25: 2, dj: dj + 25: 2]
                        nc.tensor.matmul(ps[:],
                                         w2bbd[:, 128 * t:128 * (t + 1)],
                                         rhs, start=(t == 0), stop=(t == 8))
                    for g2 in range(2):
                        gb = 64 * (2 * r + g2) + BSB * sb
                        stag = stp.tile([64, BSB * 169], bf16, tag="st")
                        nc.scalar.activation(stag[:],
                                             ps[64 * g2:64 * (g2 + 1), :],
                                             Tanh,
                                             bias=b2bt[64 * g2:64 * (g2 + 1)])
                        sv = stag[:].rearrange("c (b p) -> c b p", b=BSB)
                        nc.vector.tensor_copy(h2b_bv[:, gb:gb + BSB, :], sv)
                        nc.vector.tensor_copy(h2b_rv[:, gb:gb + BSB, :], sv)

            xq = [x_dma(0), x_dma(1)]
            h1_cur = conv1(xq.pop(0))
            for sb in range(nsb):
                if sb + 2 < nsb:
                    xq.append(x_dma(sb + 2))
                h1_next = conv1(xq.pop(0)) if sb + 1 < nsb else None
                h2a_t = conv2a(h1_cur)
                conv2b(sb, h2a_t)
                h1_cur = h1_next

            # ================= phase 2: locally-connected stack =============
            # Each matmul covers 2 output positions (j0, j0+1) x 64co on M
            # and 2 input columns (c, c+1) x 64ci on K (replica rows supply
            # col c+1), so the full 128x128 array streams N=256 images.
            # Replica rows are written per-position right after each ACT.
            PAD = 256

            def lconv(Ho, Wo, kh, ncp, Wi, lw_d, src_rep, dst_write):
                groups = [(i, j0) for i in range(Ho) for j0 in range(0, Wo, 2)]
                PF = 3
                tq = []

                def issue(gi):
                    # one big dma per group: [128, kh*ncp*128] (contiguous
                    # 3.75KB/partition) fans out across all 16 SDMA engines
                    t = lwp.tile([128, 1920], bf16, tag="lwn")
                    nc.sync.dma_start(t[:, :kh * ncp * 128], lw_d[gi])
                    tq.append(t)

                for gi in range(min(PF, len(groups))):
                    issue(gi)
                for gi, (i, j0) in enumerate(groups):
                    if gi + PF < len(groups):
                        issue(gi + PF)
                    gt = tq[gi]
                    ps = lps.tile([128, 256], f32, tag="lps")
                    n, last = 0, kh * ncp - 1
                    for u in range(kh):
                        for q in range(ncp):
                            col = PAD + ((i + u) * Wi + (j0 + 2 * q)) * 256
                            b0 = (u * ncp + q) * 128
                            nc.tensor.matmul(ps[:], gt[:, b0:b0 + 128],
                                             src_rep[:, col:col + 256],
                                             start=(n == 0), stop=(n == last))
                            n += 1
                    dst_write(i, j0, ps)

            h3rep = big2.tile([128, 256 + 81 * 256], bf16, tag="b2")
            nc.gpsimd.memset(h3rep[64:128, 81 * 256:82 * 256], 0.0)

            def wr3(i, j0, ps):
                for t in range(2):
                    j = j0 + t
                    if j >= 9:
                        break
                    p = 9 * i + j
                    d0, d1 = PAD + 256 * p, PAD + 256 * (p + 1)
                    nc.scalar.activation(h3rep[0:64, d0:d1],
                                         ps[64 * t:64 * (t + 1), :], Tanh,
                                         bias=lb3t[:, p:p + 1])
                    nc.vector.tensor_copy(h3rep[64:128, 256 * p:256 * (p + 1)],
                                          h3rep[0:64, d0:d1])

            # ---- lconv3: 13x13 -> 9x9, 5x5 taps ----
            lconv(9, 9, 5, 3, 13, lw3n_d, h2brep, wr3)

            # h2brep is dead now; h4rep reuses its slot
            h4rep = big1.tile([128, 256 + 25 * 256], bf16, tag="b1")
            nc.gpsimd.memset(h4rep[64:128, 25 * 256:26 * 256], 0.0)

            def wr4(i, j0, ps):
                for t in range(2):
                    j = j0 + t
                    if j >= 5:
                        break
                    p = 5 * i + j
                    d0, d1 = PAD + 256 * p, PAD + 256 * (p + 1)
                    nc.scalar.activation(h4rep[0:64, d0:d1],
                                         ps[64 * t:64 * (t + 1), :], Tanh,
                                         bias=lb4t[:, p:p + 1])
                    nc.vector.tensor_copy(h4rep[64:128, 256 * p:256 * (p + 1)],
                                          h4rep[0:64, d0:d1])

            # ---- lconv4: 9x9 -> 5x5, 5x5 taps ----
            lconv(5, 5, 5, 3, 9, lw4n_d, h3rep, wr4)

            # h3rep is dead after lconv4; h5t reuses big2's slot
            h5t = big2.tile([64, 9 * 256], bf16, tag="b2")

            def wr5(i, j0, ps):
                for t in range(2):
                    j = j0 + t
                    if j >= 3:
                        break
                    p = 3 * i + j
                    nc.scalar.activation(h5t[:, 256 * p:256 * (p + 1)],
                                         ps[64 * t:64 * (t + 1), :], Tanh,
                                         bias=lb5t[:, p:p + 1])

            # ---- lconv5: 5x5 -> 3x3, 3x3 taps ----
            lconv(3, 3, 3, 2, 5, lw5n_d, h4rep, wr5)

            # ---- head: logits[o, b] = sum_f hw[o, f] feat[f, b] ----
            psh = hps.tile([2, 256], f32)
            for yx in range(9):
                nc.tensor.matmul(psh[:], hwch[:, 2 * yx:2 * yx + 2],
                                 h5t[:, 256 * yx:256 * (yx + 1)],
                                 start=(yx == 0), stop=(yx == 8))
            lg = wp.tile([2, 256], f32)
            nc.vector.tensor_copy(lg[:], psh[:])
            nc.sync.dma_start(logits_d[:], lg[:])

    nc.compile()
    return nc


def _prep_weights(w1, b1, w2a, b2a, w2b, b2b, lw3, lb3, lw4, lb4, lw5, lb5, hw):
    """Host-side reshape of weights into the on-chip matmul layouts."""
    out = {}
    # conv1: rows 20*di + 5*g + ci ; cols dj*128 + 32*g + co
    w1di = np.zeros((60, 3, 128), np.float32)
    for dj in range(3):
        for di in range(3):
            blk = w1[:, :, di, dj].T          # [5ci, 32co]
            for g in range(4):
                w1di[20 * di + 5 * g:20 * di + 5 * g + 5, dj,
                     32 * g:32 * g + 32] = blk
    out["w1di"] = w1di.reshape(60, 3 * 128).astype(BF)
    out["b1t"] = np.tile(b1, 4)[:, None].astype(np.float32)

    # conv2a pairs: rows (32*g2 + ci) -> dj=0, rows (64 + 32*g2 + ci) -> dj=1
    w2ap = np.zeros((128, 3, 128), np.float32)
    w2as = np.zeros((64, 3, 128), np.float32)
    for di in range(3):
        b0 = w2a[:, :, di, 0].T               # [32ci, 64co]
        b1_ = w2a[:, :, di, 1].T
        b2_ = w2a[:, :, di, 2].T
        for g2 in range(2):
            w2ap[32 * g2:32 * (g2 + 1), di, 64 * g2:64 * (g2 + 1)] = b0
            w2ap[64 + 32 * g2:64 + 32 * (g2 + 1), di,
                 64 * g2:64 * (g2 + 1)] = b1_
            w2as[32 * g2:32 * (g2 + 1), di, 64 * g2:64 * (g2 + 1)] = b2_
    out["w2ap"] = w2ap.reshape(128, 3 * 128).astype(BF)
    out["w2as"] = w2as.reshape(64, 3 * 128).astype(BF)
    out["b2at"] = np.tile(b2a, 2)[:, None].astype(np.float32)

    TAPS3 = [(di, dj) for di in range(3) for dj in range(3)]
    w2bbd = np.zeros((128, 9, 128), np.float32)
    for t, (di, dj) in enumerate(TAPS3):
        blk = w2b[:, :, di, dj].T             # [64ci, 64co]
        for g2 in range(2):
            w2bbd[64 * g2:64 * (g2 + 1), t, 64 * g2:64 * (g2 + 1)] = blk
    out["w2bbd"] = w2bbd.reshape(128, 9 * 128).astype(BF)
    out["b2bt"] = np.tile(b2b, 2)[:, None].astype(np.float32)

    def lc_pack2(lw, Ho, Wo, kh, kw):
        # per (i, j0) group: kh u-rows x ncp col-pair stationaries [128, 128]
        # K row 64v+ci = input col c+v (v=1 from replica rows); M col
        # 64t+co = output position (i, j0+t).
        ncp = (kw + 1) // 2
        j0s = list(range(0, Wo, 2))
        out_w = np.zeros((Ho * len(j0s), 128, kh * ncp * 128), np.float32)
        g = 0
        for i in range(Ho):
            for j0 in j0s:
                for u in range(kh):
                    for q in range(ncp):
                        c = j0 + 2 * q
                        b0 = (u * ncp + q) * 128
                        for v in range(2):
                            for t in range(2):
                                j = j0 + t
                                if j >= Wo:
                                    continue
                                tap = c + v - j
                                if 0 <= tap < kw:
                                    out_w[g, 64 * v:64 * v + 64,
                                          b0 + 64 * t:b0 + 64 * t + 64] \
                                        = lw[i, j, :, :, u, tap].T
                g += 1
        return out_w.astype(BF)

    out["lw3n"] = lc_pack2(lw3, 9, 9, 5, 5)
    out["lb3t"] = np.ascontiguousarray(
        lb3.transpose(2, 0, 1).reshape(64, 81)).astype(np.float32)
    out["lw4n"] = lc_pack2(lw4, 5, 5, 5, 5)
    out["lb4t"] = np.ascontiguousarray(
        lb4.transpose(2, 0, 1).reshape(64, 25)).astype(np.float32)
    out["lw5n"] = lc_pack2(lw5, 3, 3, 3, 3)
    out["lb5t"] = np.ascontiguousarray(
        lb5.transpose(2, 0, 1).reshape(64, 9)).astype(np.float32)

    out["hwch"] = np.ascontiguousarray(
        hw[:, :576].reshape(2, 64, 9).transpose(1, 2, 0).reshape(64, 18)
    ).astype(BF)
    return out


def kernel(x, info, w1, b1, w2a, b2a, w2b, b2b, lw3, lb3, lw4, lb4, lw5, lb5,
           hw, hb, _trace=False):
    x = np.asarray(x, np.float32)
    if "nc" not in _CACHE:
        _CACHE["nc"] = _build_module()
    nc = _CACHE["nc"]

    wts = _prep_weights(np.asarray(w1, np.float32), np.asarray(b1, np.float32),
                        np.asarray(w2a, np.float32), np.asarray(b2a, np.float32),
                        np.asarray(w2b, np.float32), np.asarray(b2b, np.float32),
                        np.asarray(lw3, np.float32), np.asarray(lb3, np.float32),
                        np.asarray(lw4, np.float32), np.asarray(lb4, np.float32),
                        np.asarray(lw5, np.float32), np.asarray(lb5, np.float32),
                        np.asarray(hw, np.float32))

    xb = np.ascontiguousarray(x.reshape(B_FULL, 5, 3600)).astype(BF)
    in_maps = []
    for c in range(N_CORES):
        m = dict(wts)
        m["x"] = xb[c * B_CORE:(c + 1) * B_CORE]
        in_maps.append(m)

    res = bass_utils.run_bass_kernel_spmd(
        nc, in_maps, core_ids=list(range(N_CORES)), trace=_trace)
    if _trace:
        _CACHE["last_results"] = res

    logits = np.concatenate(
        [res.results[c]["logits"].T for c in range(N_CORES)], axis=0)  # [2048, 2]

    # host-side tail: info contribution + bias + softmax (fp32)
    info = np.asarray(info, np.float32)
    hw = np.asarray(hw, np.float32)
    hb = np.asarray(hb, np.float32)
    logits = logits + info @ hw[:, 576:].T + hb[None, :]
    m = logits.max(axis=1, keepdims=True)
    e = np.exp(logits - m)
    return (e / e.sum(axis=1, keepdims=True)).astype(np.float32)



# revision 16
# speedup vs baseline: 1.2460x; 1.2460x over previous
"""Trainium2 Bass kernel for the DeepFace-style CNN (nn_DeepFace_10574209482846).

Sharding: pure data parallel - batch 2048 split as 256 images per core
across 8 cores; all weights replicated (host-preprocessed into matmul-
friendly layouts, cast to bf16).

v3: software-pipelined phase 1 (conv1 of sb+1 ahead of conv2a of sb,
so the PE never drains on conv1's activations and the HAM clock stays
warm) + conv2b tanh into contiguous staging with the (y,x,b) scatter
moved to the Vector engine (writes base + replica rows directly).

v2: tap-packed K layouts to cut PE stream cycles:
 - conv1: (grp, ci, di) packed on K (K=60), 3 accumulating matmuls (dj)
   over di-shifted x replicas loaded straight from DRAM.
 - conv2a: 2-group pairs with a +1-column replica on partitions 64..127
   (K=128): 3 pair-matmuls (dj=0,1) + 3 singles (dj=2, K=64).
 - conv2b: 9 taps, K=128 2-group block-diagonal (unchanged).
 - locally-connected stack: batch-contiguous (y, x, b) layout with
   +1-position replica rows, tap-pair matmuls (unchanged).
All inter-layer pools double-buffered so sub-batches pipeline with no
PE gaps (keeps the PE HAM clock at 2.4 GHz).
"""

import numpy as np
import concourse.bass as bass
import concourse.bacc as bacc
import concourse.tile as tile
import concourse.mybir as mybir
from concourse import bass_utils

bf16 = mybir.dt.bfloat16
f32 = mybir.dt.float32
BF = mybir.dt.np(bf16)

N_CORES = 8
B_FULL = 2048
B_CORE = 256          # images per core
SB = 8                # images per sub-batch (2 per group)
NSB = B_CORE // SB    # 32
BSB = SB // 4         # 2 images per group per sub-batch

L1 = BSB * 841        # h1 elements per pair tile (flat (b, 29, 29))

_CACHE = {}


def _build_module(nsb=NSB):
    nc = bacc.Bacc("TRN2", target_bir_lowering=False, debug=False,
                   enable_asserts=True, num_devices=N_CORES)

    # ---- DRAM I/O ----
    x_d = nc.dram_tensor("x", [B_CORE, 5, 3600], bf16, kind="ExternalInput").ap()
    w1di_d = nc.dram_tensor("w1di", [60, 3 * 128], bf16, kind="ExternalInput").ap()
    b1t_d = nc.dram_tensor("b1t", [128, 1], f32, kind="ExternalInput").ap()
    w2ap_d = nc.dram_tensor("w2ap", [128, 3 * 128], bf16, kind="ExternalInput").ap()
    w2as_d = nc.dram_tensor("w2as", [64, 3 * 128], bf16, kind="ExternalInput").ap()
    b2at_d = nc.dram_tensor("b2at", [128, 1], f32, kind="ExternalInput").ap()
    w2bbd_d = nc.dram_tensor("w2bbd", [128, 9 * 128], bf16, kind="ExternalInput").ap()
    b2bt_d = nc.dram_tensor("b2bt", [128, 1], f32, kind="ExternalInput").ap()
    lw3n_d = nc.dram_tensor("lw3n", [45, 128, 1920], bf16, kind="ExternalInput").ap()
    lb3_d = nc.dram_tensor("lb3t", [64, 81], f32, kind="ExternalInput").ap()
    lw4n_d = nc.dram_tensor("lw4n", [15, 128, 1920], bf16, kind="ExternalInput").ap()
    lb4_d = nc.dram_tensor("lb4t", [64, 25], f32, kind="ExternalInput").ap()
    lw5n_d = nc.dram_tensor("lw5n", [6, 128, 768], bf16, kind="ExternalInput").ap()
    lb5_d = nc.dram_tensor("lb5t", [64, 9], f32, kind="ExternalInput").ap()
    hwch_d = nc.dram_tensor("hwch", [64, 18], bf16, kind="ExternalInput").ap()
    logits_d = nc.dram_tensor("logits", [2, B_CORE], f32, kind="ExternalOutput").ap()

    Tanh = mybir.ActivationFunctionType.Tanh

    with tile.TileContext(nc) as tc:
        with (
            tc.tile_pool(name="wp", bufs=1) as wp,
            tc.tile_pool(name="lwp", bufs=4) as lwp,
            tc.tile_pool(name="xp", bufs=2) as xp,
            tc.tile_pool(name="h1p", bufs=2) as h1p,
            tc.tile_pool(name="h2ap", bufs=2) as h2ap,
            tc.tile_pool(name="stp", bufs=4) as stp,
            tc.tile_pool(name="big1", bufs=1) as big1,
            tc.tile_pool(name="big2", bufs=1) as big2,
            tc.tile_pool(name="cps", bufs=5, space="PSUM") as cps,
            tc.tile_pool(name="lps", bufs=2, space="PSUM") as lps,
            tc.tile_pool(name="hps", bufs=1, space="PSUM") as hps,
        ):
            # ---- persistent weights ----
            w1di = wp.tile([60, 3 * 128], bf16)
            nc.sync.dma_start(w1di[:], w1di_d[:])
            b1t = wp.tile([128, 1], f32)
            nc.sync.dma_start(b1t[:], b1t_d[:])
            w2ap = wp.tile([128, 3 * 128], bf16)
            nc.sync.dma_start(w2ap[:], w2ap_d[:])
            w2as = wp.tile([64, 3 * 128], bf16)
            nc.sync.dma_start(w2as[:], w2as_d[:])
            b2at = wp.tile([128, 1], f32)
            nc.sync.dma_start(b2at[:], b2at_d[:])
            w2bbd = wp.tile([128, 9 * 128], bf16)
            nc.sync.dma_start(w2bbd[:], w2bbd_d[:])
            b2bt = wp.tile([128, 1], f32)
            nc.sync.dma_start(b2bt[:], b2bt_d[:])
            lb3t = wp.tile([64, 81], f32)
            nc.sync.dma_start(lb3t[:], lb3_d[:])
            lb4t = wp.tile([64, 25], f32)
            nc.sync.dma_start(lb4t[:], lb4_d[:])
            lb5t = wp.tile([64, 9], f32)
            nc.sync.dma_start(lb5t[:], lb5_d[:])
            hwch = wp.tile([64, 18], bf16)
            nc.sync.dma_start(hwch[:], hwch_d[:])

            # ---- persistent activations, batch-contiguous (y, x, b) with a
            # 256-col leading pad; rows 64-127 hold the +1-position replica
            # (element e lives at col PAD+e on rows 0-63 and col e on 64-127).
            # h4rep/h5t reuse the big slots once h2brep/h3rep are dead.
            h2brep = big1.tile([128, 256 + 169 * 256], bf16, tag="b1")
            # zero the one-past-the-end replica column block (read with zero
            # weights by the edge position-pair matmuls; must not be NaN)
            nc.gpsimd.memset(h2brep[64:128, 169 * 256:170 * 256], 0.0)

            # (c, b, p) views of base and replica halves, p = y*13 + x
            h2b_bv = h2brep[0:64, 256:].rearrange("c (p b) -> c b p",
                                                  p=169, b=256)
            h2b_rv = h2brep[64:128, 0:169 * 256].rearrange("c (p b) -> c b p",
                                                           p=169, b=256)

            # ======== phase 1: conv1 -> conv2a -> conv2b, sw-pipelined =====
            # conv1 of sub-batch sb+1 issues before conv2a of sb so the PE
            # never waits on conv1's activation drain (and the HAM clock
            # stays warm); conv2b tanh lands in a contiguous staging tile,
            # the (y,x,b) scatter runs on the idle Vector engine (both base
            # and +1-position replica rows, so no bulk replica copy later).
            def x_dma(sb):
                # x tile: rows 20*di + 5*g + ci hold x[ci] shifted di rows up
                x3 = xp.tile([60, BSB * 3600], bf16, tag="x")
                for di in range(3):
                    for g in range(4):
                        b0 = 64 * g + BSB * sb
                        src = x_d[b0:b0 + BSB, :, 60 * di:].rearrange(
                            "b c m -> c b m")
                        dst = x3[20 * di + 5 * g:20 * di + 5 * g + 5,
                                 :].rearrange(
                            "c (b m) -> c b m", b=BSB)[:, :, :3600 - 60 * di]
                        nc.sync.dma_start(dst, src)
                return x3

            def conv1(x3):
                # K=60 (4 groups x 5ci x 3di), 3 dj-matmuls
                xv = x3[:].rearrange("c (b h w) -> c b h w", b=BSB, h=60, w=60)
                h1pr = {}
                for r in range(2):
                    h1pr[r] = h1p.tile([128, 1 + L1 + 3], bf16, tag=f"h1{r}",
                                       name=f"h1pair{r}")
                for (y0, ny) in [(0, 8), (8, 8), (16, 8), (24, 5)]:
                    ps = cps.tile([128, BSB * 8 * 29], f32, tag="cps")
                    psw = ps[:, :BSB * ny * 29]
                    for dj in range(3):
                        rhs = xv[:, :, 2 * y0: 2 * y0 + 2 * ny - 1: 2,
                                 dj: dj + 57: 2]
                        nc.tensor.matmul(psw, w1di[:, 128 * dj:128 * (dj + 1)],
                                         rhs, start=(dj == 0), stop=(dj == 2))
                    psv = psw.rearrange("c (b y x) -> c b y x",
                                        b=BSB, y=ny, x=29)
                    for r in range(2):
                        dstv = h1pr[r][0:64, 1:1 + L1].rearrange(
                            "c (b h w) -> c b h w", b=BSB, h=29, w=29)
                        nc.scalar.activation(dstv[:, :, y0:y0 + ny, :],
                                             psv[64 * r:64 * (r + 1)],
                                             Tanh, bias=b1t[0:64])
                # +1-element replica rows (gives dj+1 taps in pair matmuls)
                for r in range(2):
                    nc.vector.tensor_copy(h1pr[r][64:128, 0:L1],
                                          h1pr[r][0:64, 1:1 + L1])
                return h1pr

            def conv2a(h1pr):
                # per pair, 3 pair-matmuls (K=128) + 3 singles
                h2a_t = {}
                for r in range(2):
                    h2a_t[r] = h2ap.tile([128, BSB * 729], bf16, tag=f"h2a{r}",
                                         name=f"h2a{r}")
                    h2av = h2a_t[r][:].rearrange("c (b h w) -> c b h w",
                                                 b=BSB, h=27, w=27)
                    basev = h1pr[r][:, 1:1 + L1].rearrange(
                        "c (b h w) -> c b h w", b=BSB, h=29, w=29)
                    sglv = h1pr[r][0:64, 3:3 + L1].rearrange(
                        "c (b h w) -> c b h w", b=BSB, h=29, w=29)
                    for (y0, ny) in [(0, 9), (9, 9), (18, 9)]:
                        ps = cps.tile([128, BSB * 9 * 27], f32, tag="cps")
                        psw = ps[:, :BSB * ny * 27]
                        for di in range(3):
                            rhs = basev[:, :, y0 + di: y0 + di + ny, 0:27]
                            nc.tensor.matmul(
                                psw, w2ap[:, 128 * di:128 * (di + 1)], rhs,
                                start=(di == 0), stop=False)
                        for di in range(3):
                            rhs = sglv[:, :, y0 + di: y0 + di + ny, 0:27]
                            nc.tensor.matmul(
                                psw, w2as[:, 128 * di:128 * (di + 1)], rhs,
                                start=False, stop=(di == 2))
                        nc.scalar.activation(h2av[:, :, y0:y0 + ny, :], psw,
                                             Tanh, bias=b2at[:])
                return h2a_t

            TAPS3 = [(di, dj) for di in range(3) for dj in range(3)]

            def conv2b(sb, h2a_t):
                # per pair, K=128 block-diag, stride 2
                for r in range(2):
                    h2av = h2a_t[r][:].rearrange("c (b h w) -> c b h w",
                                                 b=BSB, h=27, w=27)
                    ps = cps.tile([128, BSB * 169], f32, tag="cps")
                    for t, (di, dj) in enumerate(TAPS3):
                        rhs = h2av[:, :, di: di + 25: 2, dj: dj + 25: 2]
                        nc.tensor.matmul(ps[:],
                                         w2bbd[:, 128 * t:128 * (t + 1)],
                                         rhs, start=(t == 0), stop=(t == 8))
                    for g2 in range(2):
                        gb = 64 * (2 * r + g2) + BSB * sb
                        stag = stp.tile([64, BSB * 169], bf16, tag="st")
                        nc.scalar.activation(stag[:],
                                             ps[64 * g2:64 * (g2 + 1), :],
                                             Tanh,
                                             bias=b2bt[64 * g2:64 * (g2 + 1)])
                        sv = stag[:].rearrange("c (b p) -> c b p", b=BSB)
                        nc.vector.tensor_copy(h2b_bv[:, gb:gb + BSB, :], sv)
                        nc.vector.tensor_copy(h2b_rv[:, gb:gb + BSB, :], sv)

            xq = [x_dma(0), x_dma(1)]
            h1_cur = conv1(xq.pop(0))
            for sb in range(nsb):
                if sb + 2 < nsb:
                    xq.append(x_dma(sb + 2))
                h1_next = conv1(xq.pop(0)) if sb + 1 < nsb else None
                h2a_t = conv2a(h1_cur)
                conv2b(sb, h2a_t)
                h1_cur = h1_next

            # ================= phase 2: locally-connected stack =============
            # Each matmul covers 2 output positions (j0, j0+1) x 64co on M
            # and 2 input columns (c, c+1) x 64ci on K (replica rows supply
            # col c+1), so the full 128x128 array streams N=256 images.
            # Replica rows are written per-position right after each ACT.
            PAD = 256

            def lconv(Ho, Wo, kh, ncp, Wi, lw_d, src_rep, dst_write):
                groups = [(i, j0) for i in range(Ho) for j0 in range(0, Wo, 2)]
                PF = 3
                tq = []

                def issue(gi):
                    # one big dma per group: [128, kh*ncp*128] (contiguous
                    # 3.75KB/partition) fans out across all 16 SDMA engines
                    t = lwp.tile([128, 1920], bf16, tag="lwn")
                    nc.sync.dma_start(t[:, :kh * ncp * 128], lw_d[gi])
                    tq.append(t)

                for gi in range(min(PF, len(groups))):
                    issue(gi)
                for gi, (i, j0) in enumerate(groups):
                    if gi + PF < len(groups):
                        issue(gi + PF)
                    gt = tq[gi]
                    ps = lps.tile([128, 256], f32, tag="lps")
                    n, last = 0, kh * ncp - 1
                    for u in range(kh):
                        for q in range(ncp):
                            col = PAD + ((i + u) * Wi + (j0 + 2 * q)) * 256
                            b0 = (u * ncp + q) * 128
                            nc.tensor.matmul(ps[:], gt[:, b0:b0 + 128],
                                             src_rep[:, col:col + 256],
                                             start=(n == 0), stop=(n == last))
                            n += 1
                    dst_write(i, j0, ps)

            h3rep = big2.tile([128, 256 + 81 * 256], bf16, tag="b2")
            nc.gpsimd.memset(h3rep[64:128, 81 * 256:82 * 256], 0.0)

            def wr3(i, j0, ps):
                for t in range(2):
                    j = j0 + t
                    if j >= 9:
                        break
                    p = 9 * i + j
                    d0, d1 = PAD + 256 * p, PAD + 256 * (p + 1)
                    nc.scalar.activation(h3rep[0:64, d0:d1],
                                         ps[64 * t:64 * (t + 1), :], Tanh,
                                         bias=lb3t[:, p:p + 1])
                    nc.vector.tensor_copy(h3rep[64:128, 256 * p:256 * (p + 1)],
                                          h3rep[0:64, d0:d1])

            # ---- lconv3: 13x13 -> 9x9, 5x5 taps ----
            lconv(9, 9, 5, 3, 13, lw3n_d, h2brep, wr3)

            # h2brep is dead now; h4rep reuses its slot
            h4rep = big1.tile([128, 256 + 25 * 256], bf16, tag="b1")
            nc.gpsimd.memset(h4rep[64:128, 25 * 256:26 * 256], 0.0)

            def wr4(i, j0, ps):
                for t in range(2):
                    j = j0 + t
                    if j >= 5:
                        break
                    p = 5 * i + j
                    d0, d1 = PAD + 256 * p, PAD + 256 * (p + 1)
                    nc.scalar.activation(h4rep[0:64, d0:d1],
                                         ps[64 * t:64 * (t + 1), :], Tanh,
                                         bias=lb4t[:, p:p + 1])
                    nc.vector.tensor_copy(h4rep[64:128, 256 * p:256 * (p + 1)],
                                          h4rep[0:64, d0:d1])

            # ---- lconv4: 9x9 -> 5x5, 5x5 taps ----
            lconv(5, 5, 5, 3, 9, lw4n_d, h3rep, wr4)

            # h3rep is dead after lconv4; h5t reuses big2's slot
            h5t = big2.tile([64, 9 * 256], bf16, tag="b2")

            def wr5(i, j0, ps):
                for t in range(2):
                    j = j0 + t
                    if j >= 3:
                        break
                    p = 3 * i + j
                    nc.scalar.activation(h5t[:, 256 * p:256 * (p + 1)],
                                         ps[64 * t:64 * (t + 1), :], Tanh,
                                         bias=lb5t[:, p:p + 1])

            # ---- lconv5: 5x5 -> 3x3, 3x3 taps ----
            lconv(3, 3, 3, 2, 5, lw5n_d, h4rep, wr5)

            # ---- head: logits[o, b] = sum_f hw[o, f] feat[f, b] ----
            psh = hps.tile([2, 256], f32)
            for yx in range(9):
                nc.tensor.matmul(psh[:], hwch[:, 2 * yx:2 * yx + 2],
                                 h5t[:, 256 * yx:256 * (yx + 1)],
                                 start=(yx == 0), stop=(yx == 8))
            lg = wp.tile([2, 256], f32)
            nc.vector.tensor_copy(lg[:], psh[:])
            nc.sync.dma_start(logits_d[:], lg[:])

    nc.compile()
    return nc


def _prep_weights(w1, b1, w2a, b2a, w2b, b2b, lw3, lb3, lw4, lb4, lw5, lb5, hw):
    """Host-side reshape of weights into the on-chip matmul layouts."""
    out = {}
    # conv1: rows 20*di + 5*g + ci ; cols dj*128 + 32*g + co
    w1di = np.zeros((60, 3, 128), np.float32)
    for dj in range(3):
        for di in range(3):
            blk = w1[:, :, di, dj].T          # [5ci, 32co]
            for g in range(4):
                w1di[20 * di + 5 * g:20 * di + 5 * g + 5, dj,
                     32 * g:32 * g + 32] = blk
    out["w1di"] = w1di.reshape(60, 3 * 128).astype(BF)
    out["b1t"] = np.tile(b1, 4)[:, None].astype(np.float32)

    # conv2a pairs: rows (32*g2 + ci) -> dj=0, rows (64 + 32*g2 + ci) -> dj=1
    w2ap = np.zeros((128, 3, 128), np.float32)
    w2as = np.zeros((64, 3, 128), np.float32)
    for di in range(3):
        b0 = w2a[:, :, di, 0].T               # [32ci, 64co]
        b1_ = w2a[:, :, di, 1].T
        b2_ = w2a[:, :, di, 2].T
        for g2 in range(2):
            w2ap[32 * g2:32 * (g2 + 1), di, 64 * g2:64 * (g2 + 1)] = b0
            w2ap[64 + 32 * g2:64 + 32 * (g2 + 1), di,
                 64 * g2:64 * (g2 + 1)] = b1_
            w2as[32 * g2:32 * (g2 + 1), di, 64 * g2:64 * (g2 + 1)] = b2_
    out["w2ap"] = w2ap.reshape(128, 3 * 128).astype(BF)
    out["w2as"] = w2as.reshape(64, 3 * 128).astype(BF)
    out["b2at"] = np.tile(b2a, 2)[:, None].astype(np.float32)

    TAPS3 = [(di, dj) for di in range(3) for dj in range(3)]
    w2bbd = np.zeros((128, 9, 128), np.float32)
    for t, (di, dj) in enumerate(TAPS3):
        blk = w2b[:, :, di, dj].T             # [64ci, 64co]
        for g2 in range(2):
            w2bbd[64 * g2:64 * (g2 + 1), t, 64 * g2:64 * (g2 + 1)] = blk
    out["w2bbd"] = w2bbd.reshape(128, 9 * 128).astype(BF)
    out["b2bt"] = np.tile(b2b, 2)[:, None].astype(np.float32)

    def lc_pack2(lw, Ho, Wo, kh, kw):
        # per (i, j0) group: kh u-rows x ncp col-pair stationaries [128, 128]
        # K row 64v+ci = input col c+v (v=1 from replica rows); M col
        # 64t+co = output position (i, j0+t).
        ncp = (kw + 1) // 2
        j0s = list(range(0, Wo, 2))
        out_w = np.zeros((Ho * len(j0s), 128, kh * ncp * 128), np.float32)
        g = 0
        for i in range(Ho):
            for j0 in j0s:
                for u in range(kh):
                    for q in range(ncp):
                        c = j0 + 2 * q
                        b0 = (u * ncp + q) * 128
                        for v in range(2):
                            for t in range(2):
                                j = j0 + t
                                if j >= Wo:
                                    continue
                                tap = c + v - j
                                if 0 <= tap < kw:
                                    out_w[g, 64 * v:64 * v + 64,
                                          b0 + 64 * t:b0 + 64 * t + 64] \
                                        = lw[i, j, :, :, u, tap].T
                g += 1
        return out_w.astype(BF)

    out["lw3n"] = lc_pack2(lw3, 9, 9, 5, 5)
    out["lb3t"] = np.ascontiguousarray(
        lb3.transpose(2, 0, 1).reshape(64, 81)).astype(np.float32)
    out["lw4n"] = lc_pack2(lw4, 5, 5, 5, 5)
    out["lb4t"] = np.ascontiguousarray(
        lb4.transpose(2, 0, 1).reshape(64, 25)).astype(np.float32)
    out["lw5n"] = lc_pack2(lw5, 3, 3, 3, 3)
    out["lb5t"] = np.ascontiguousarray(
        lb5.transpose(2, 0, 1).reshape(64, 9)).astype(np.float32)

    out["hwch"] = np.ascontiguousarray(
        hw[:, :576].reshape(2, 64, 9).transpose(1, 2, 0).reshape(64, 18)
    ).astype(BF)
    return out


def kernel(x, info, w1, b1, w2a, b2a, w2b, b2b, lw3, lb3, lw4, lb4, lw5, lb5,
           hw, hb, _trace=False):
    x = np.asarray(x, np.float32)
    if "nc" not in _CACHE:
        _CACHE["nc"] = _build_module()
    nc = _CACHE["nc"]

    wts = _prep_weights(np.asarray(w1, np.float32), np.asarray(b1, np.float32),
                        np.asarray(w2a, np.float32), np.asarray(b2a, np.float32),
                        np.asarray(w2b, np.float32), np.asarray(b2b, np.float32),
                        np.asarray(lw3, np.float32), np.asarray(lb3, np.float32),
                        np.asarray(lw4, np.float32), np.asarray(lb4, np.float32),
                        np.asarray(lw5, np.float32), np.asarray(lb5, np.float32),
                        np.asarray(hw, np.float32))

    xb = np.ascontiguousarray(x.reshape(B_FULL, 5, 3600)).astype(BF)
    in_maps = []
    for c in range(N_CORES):
        m = dict(wts)
        m["x"] = xb[c * B_CORE:(c + 1) * B_CORE]
        in_maps.append(m)

    res = bass_utils.run_bass_kernel_spmd(
        nc, in_maps, core_ids=list(range(N_CORES)), trace=_trace)
    if _trace:
        _CACHE["last_results"] = res

    logits = np.concatenate(
        [res.results[c]["logits"].T for c in range(N_CORES)], axis=0)  # [2048, 2]

    # host-side tail: info contribution + bias + softmax (fp32)
    info = np.asarray(info, np.float32)
    hw = np.asarray(hw, np.float32)
    hb = np.asarray(hb, np.float32)
    logits = logits + info @ hw[:, 576:].T + hb[None, :]
    m = logits.max(axis=1, keepdims=True)
    e = np.exp(logits - m)
    return (e / e.sum(axis=1, keepdims=True)).astype(np.float32)



# revision 29
# speedup vs baseline: 1.2649x; 1.0152x over previous
"""Trainium2 Bass kernel for the DeepFace-style CNN (nn_DeepFace_10574209482846).

Sharding: pure data parallel - batch 2048 split as 256 images per core
across 8 cores; all weights replicated (host-preprocessed into matmul-
friendly layouts, cast to bf16).

v3: software-pipelined phase 1 (conv1 of sb+1 ahead of conv2a of sb,
so the PE never drains on conv1's activations and the HAM clock stays
warm) + conv2b tanh into contiguous staging with the (y,x,b) scatter
moved to the Vector engine (writes base + replica rows directly).

v2: tap-packed K layouts to cut PE stream cycles:
 - conv1: (grp, ci, di) packed on K (K=60), 3 accumulating matmuls (dj)
   over di-shifted x replicas loaded straight from DRAM.
 - conv2a: 2-group pairs with a +1-column replica on partitions 64..127
   (K=128): 3 pair-matmuls (dj=0,1) + 3 singles (dj=2, K=64).
 - conv2b: 9 taps, K=128 2-group block-diagonal (unchanged).
 - locally-connected stack: batch-contiguous (y, x, b) layout with
   +1-position replica rows, tap-pair matmuls (unchanged).
All inter-layer pools double-buffered so sub-batches pipeline with no
PE gaps (keeps the PE HAM clock at 2.4 GHz).
"""

import numpy as np
import concourse.bass as bass
import concourse.bacc as bacc
import concourse.tile as tile
import concourse.mybir as mybir
from concourse import bass_utils

bf16 = mybir.dt.bfloat16
f32 = mybir.dt.float32
BF = mybir.dt.np(bf16)

N_CORES = 8
B_FULL = 2048
B_CORE = 256          # images per core
SB = 8                # images per sub-batch (2 per group)
NSB = B_CORE // SB    # 32
BSB = SB // 4         # 2 images per group per sub-batch

L1 = BSB * 841        # h1 elements per pair tile (flat (b, 29, 29))

_CACHE = {}


def _build_module(nsb=NSB):
    nc = bacc.Bacc("TRN2", target_bir_lowering=False, debug=False,
                   enable_asserts=True, num_devices=N_CORES)

    # ---- DRAM I/O ----
    x_d = nc.dram_tensor("x", [B_CORE, 5, 3600], bf16, kind="ExternalInput").ap()
    w1di_d = nc.dram_tensor("w1di", [60, 3 * 128], bf16, kind="ExternalInput").ap()
    b1t_d = nc.dram_tensor("b1t", [128, 1], f32, kind="ExternalInput").ap()
    w2ap_d = nc.dram_tensor("w2ap", [128, 3 * 128], bf16, kind="ExternalInput").ap()
    w2as_d = nc.dram_tensor("w2as", [64, 3 * 128], bf16, kind="ExternalInput").ap()
    b2at_d = nc.dram_tensor("b2at", [128, 1], f32, kind="ExternalInput").ap()
    w2bbd_d = nc.dram_tensor("w2bbd", [128, 9 * 128], bf16, kind="ExternalInput").ap()
    b2bt_d = nc.dram_tensor("b2bt", [128, 1], f32, kind="ExternalInput").ap()
    lw3n_d = nc.dram_tensor("lw3n", [45, 128, 1920], bf16, kind="ExternalInput").ap()
    lb3_d = nc.dram_tensor("lb3t", [64, 81], f32, kind="ExternalInput").ap()
    lw4n_d = nc.dram_tensor("lw4n", [15, 128, 1920], bf16, kind="ExternalInput").ap()
    lb4_d = nc.dram_tensor("lb4t", [64, 25], f32, kind="ExternalInput").ap()
    lw5n_d = nc.dram_tensor("lw5n", [6, 128, 768], bf16, kind="ExternalInput").ap()
    lb5_d = nc.dram_tensor("lb5t", [64, 9], f32, kind="ExternalInput").ap()
    hwch_d = nc.dram_tensor("hwch", [64, 18], bf16, kind="ExternalInput").ap()
    logits_d = nc.dram_tensor("logits", [2, B_CORE], f32, kind="ExternalOutput").ap()

    Tanh = mybir.ActivationFunctionType.Tanh

    with tile.TileContext(nc) as tc:
        with (
            tc.tile_pool(name="wp", bufs=1) as wp,
            tc.tile_pool(name="lwp", bufs=5) as lwp,
            tc.tile_pool(name="xp", bufs=2) as xp,
            tc.tile_pool(name="h1p", bufs=2) as h1p,
            tc.tile_pool(name="h2ap", bufs=2) as h2ap,
            tc.tile_pool(name="stp", bufs=4) as stp,
            tc.tile_pool(name="big1", bufs=1) as big1,
            tc.tile_pool(name="big2", bufs=1) as big2,
            tc.tile_pool(name="cps", bufs=5, space="PSUM") as cps,
            tc.tile_pool(name="lps", bufs=2, space="PSUM") as lps,
            tc.tile_pool(name="hps", bufs=1, space="PSUM") as hps,
        ):
            # ---- persistent weights ----
            w1di = wp.tile([60, 3 * 128], bf16)
            nc.sync.dma_start(w1di[:], w1di_d[:])
            b1t = wp.tile([128, 1], f32)
            nc.sync.dma_start(b1t[:], b1t_d[:])
            w2ap = wp.tile([128, 3 * 128], bf16)
            nc.sync.dma_start(w2ap[:], w2ap_d[:])
            w2as = wp.tile([64, 3 * 128], bf16)
            nc.sync.dma_start(w2as[:], w2as_d[:])
            b2at = wp.tile([128, 1], f32)
            nc.sync.dma_start(b2at[:], b2at_d[:])
            w2bbd = wp.tile([128, 9 * 128], bf16)
            nc.sync.dma_start(w2bbd[:], w2bbd_d[:])
            b2bt = wp.tile([128, 1], f32)
            nc.sync.dma_start(b2bt[:], b2bt_d[:])
            lb3t = wp.tile([64, 81], f32)
            nc.sync.dma_start(lb3t[:], lb3_d[:])
            lb4t = wp.tile([64, 25], f32)
            nc.sync.dma_start(lb4t[:], lb4_d[:])
            lb5t = wp.tile([64, 9], f32)
            nc.sync.dma_start(lb5t[:], lb5_d[:])
            hwch = wp.tile([64, 18], bf16)
            nc.sync.dma_start(hwch[:], hwch_d[:])

            # ---- persistent activations, batch-contiguous (y, x, b) with a
            # 256-col leading pad; rows 64-127 hold the +1-position replica
            # (element e lives at col PAD+e on rows 0-63 and col e on 64-127).
            # h4rep/h5t reuse the big slots once h2brep/h3rep are dead.
            h2brep = big1.tile([128, 256 + 169 * 256], bf16, tag="b1")
            # zero the one-past-the-end replica column block (read with zero
            # weights by the edge position-pair matmuls; must not be NaN)
            nc.gpsimd.memset(h2brep[64:128, 169 * 256:170 * 256], 0.0)

            # (c, b, p) views of base and replica halves, p = y*13 + x
            h2b_bv = h2brep[0:64, 256:].rearrange("c (p b) -> c b p",
                                                  p=169, b=256)
            h2b_rv = h2brep[64:128, 0:169 * 256].rearrange("c (p b) -> c b p",
                                                           p=169, b=256)

            # ======== phase 1: conv1 -> conv2a -> conv2b, sw-pipelined =====
            # conv1 of sub-batch sb+1 issues before conv2a of sb so the PE
            # never waits on conv1's activation drain (and the HAM clock
            # stays warm); conv2b tanh lands in a contiguous staging tile,
            # the (y,x,b) scatter runs on the idle Vector engine (both base
            # and +1-position replica rows, so no bulk replica copy later).
            def x_dma(sb):
                # x tile: rows 20*di + 5*g + ci hold x[ci] shifted di rows up
                x3 = xp.tile([60, BSB * 3600], bf16, tag="x")
                for di in range(3):
                    for g in range(4):
                        b0 = 64 * g + BSB * sb
                        src = x_d[b0:b0 + BSB, :, 60 * di:].rearrange(
                            "b c m -> c b m")
                        dst = x3[20 * di + 5 * g:20 * di + 5 * g + 5,
                                 :].rearrange(
                            "c (b m) -> c b m", b=BSB)[:, :, :3600 - 60 * di]
                        nc.sync.dma_start(dst, src)
                return x3

            def conv1(x3):
                # K=60 (4 groups x 5ci x 3di), 3 dj-matmuls
                xv = x3[:].rearrange("c (b h w) -> c b h w", b=BSB, h=60, w=60)
                h1pr = {}
                for r in range(2):
                    h1pr[r] = h1p.tile([128, 1 + L1 + 3], bf16, tag=f"h1{r}",
                                       name=f"h1pair{r}")
                for (y0, ny) in [(0, 8), (8, 8), (16, 8), (24, 5)]:
                    ps = cps.tile([128, BSB * 8 * 29], f32, tag="cps")
                    psw = ps[:, :BSB * ny * 29]
                    for dj in range(3):
                        rhs = xv[:, :, 2 * y0: 2 * y0 + 2 * ny - 1: 2,
                                 dj: dj + 57: 2]
                        nc.tensor.matmul(psw, w1di[:, 128 * dj:128 * (dj + 1)],
                                         rhs, start=(dj == 0), stop=(dj == 2))
                    psv = psw.rearrange("c (b y x) -> c b y x",
                                        b=BSB, y=ny, x=29)
                    for r in range(2):
                        dstv = h1pr[r][0:64, 1:1 + L1].rearrange(
                            "c (b h w) -> c b h w", b=BSB, h=29, w=29)
                        nc.scalar.activation(dstv[:, :, y0:y0 + ny, :],
                                             psv[64 * r:64 * (r + 1)],
                                             Tanh, bias=b1t[0:64])
                # +1-element replica rows (gives dj+1 taps in pair matmuls)
                for r in range(2):
                    nc.vector.tensor_copy(h1pr[r][64:128, 0:L1],
                                          h1pr[r][0:64, 1:1 + L1])
                return h1pr

            def conv2a(h1pr):
                # per pair, 3 pair-matmuls (K=128) + 3 singles
                h2a_t = {}
                for r in range(2):
                    h2a_t[r] = h2ap.tile([128, BSB * 729], bf16, tag=f"h2a{r}",
                                         name=f"h2a{r}")
                    h2av = h2a_t[r][:].rearrange("c (b h w) -> c b h w",
                                                 b=BSB, h=27, w=27)
                    basev = h1pr[r][:, 1:1 + L1].rearrange(
                        "c (b h w) -> c b h w", b=BSB, h=29, w=29)
                    sglv = h1pr[r][0:64, 3:3 + L1].rearrange(
                        "c (b h w) -> c b h w", b=BSB, h=29, w=29)
                    for (y0, ny) in [(0, 9), (9, 9), (18, 9)]:
                        ps = cps.tile([128, BSB * 9 * 27], f32, tag="cps")
                        psw = ps[:, :BSB * ny * 27]
                        for di in range(3):
                            rhs = basev[:, :, y0 + di: y0 + di + ny, 0:27]
                            nc.tensor.matmul(
                                psw, w2ap[:, 128 * di:128 * (di + 1)], rhs,
                                start=(di == 0), stop=False)
                        for di in range(3):
                            rhs = sglv[:, :, y0 + di: y0 + di + ny, 0:27]
                            nc.tensor.matmul(
                                psw, w2as[:, 128 * di:128 * (di + 1)], rhs,
                                start=False, stop=(di == 2))
                        nc.scalar.activation(h2av[:, :, y0:y0 + ny, :], psw,
                                             Tanh, bias=b2at[:])
                return h2a_t

            TAPS3 = [(di, dj) for di in range(3) for dj in range(3)]

            def conv2b(sb, h2a_t):
                # per pair, K=128 block-diag, stride 2
                for r in range(2):
                    h2av = h2a_t[r][:].rearrange("c (b h w) -> c b h w",
                                                 b=BSB, h=27, w=27)
                    ps = cps.tile([128, BSB * 169], f32, tag="cps")
                    for t, (di, dj) in enumerate(TAPS3):
                        rhs = h2av[:, :, di: di + 25: 2, dj: dj + 25: 2]
                        nc.tensor.matmul(ps[:],
                                         w2bbd[:, 128 * t:128 * (t + 1)],
                                         rhs, start=(t == 0), stop=(t == 8))
                    for g2 in range(2):
                        gb = 64 * (2 * r + g2) + BSB * sb
                        stag = stp.tile([64, BSB * 169], bf16, tag="st")
                        nc.scalar.activation(stag[:],
                                             ps[64 * g2:64 * (g2 + 1), :],
                                             Tanh,
                                             bias=b2bt[64 * g2:64 * (g2 + 1)])
                        sv = stag[:].rearrange("c (b p) -> c b p", b=BSB)
                        nc.vector.tensor_copy(h2b_bv[:, gb:gb + BSB, :], sv)
                        nc.vector.tensor_copy(h2b_rv[:, gb:gb + BSB, :], sv)

            # pre-issue the first lconv3 weight loads; their DMAs have no
            # deps, so they execute during phase 1 and the locally-connected
            # stack starts with a warm weight pipeline
            pre3 = []
            for _gi in range(4):
                _t = lwp.tile([128, 1920], bf16, tag="lwn")
                nc.sync.dma_start(_t[:], lw3n_d[_gi])
                pre3.append(_t)

            xq = [x_dma(0), x_dma(1)]
            h1_cur = conv1(xq.pop(0))
            for sb in range(nsb):
                if sb + 2 < nsb:
                    xq.append(x_dma(sb + 2))
                h1_next = conv1(xq.pop(0)) if sb + 1 < nsb else None
                h2a_t = conv2a(h1_cur)
                conv2b(sb, h2a_t)
                h1_cur = h1_next

            # ================= phase 2: locally-connected stack =============
            # Each matmul covers 2 output positions (j0, j0+1) x 64co on M
            # and 2 input columns (c, c+1) x 64ci on K (replica rows supply
            # col c+1), so the full 128x128 array streams N=256 images.
            # Replica rows are written per-position right after each ACT.
            PAD = 256

            def lconv(Ho, Wo, kh, ncp, Wi, lw_d, src_rep, dst_write,
                      pre=None):
                groups = [(i, j0) for i in range(Ho) for j0 in range(0, Wo, 2)]
                PF = 4
                tq = list(pre) if pre else []

                def issue(gi):
                    # one big dma per group: [128, kh*ncp*128] (contiguous
                    # 3.75KB/partition) fans out across all 16 SDMA engines
                    t = lwp.tile([128, 1920], bf16, tag="lwn")
                    nc.sync.dma_start(t[:, :kh * ncp * 128], lw_d[gi])
                    tq.append(t)

                for gi in range(len(tq), min(PF, len(groups))):
                    issue(gi)
                for gi, (i, j0) in enumerate(groups):
                    if gi + PF < len(groups):
                        issue(gi + PF)
                    gt = tq[gi]
                    ps = lps.tile([128, 256], f32, tag="lps")
                    n, last = 0, kh * ncp - 1
                    for u in range(kh):
                        for q in range(ncp):
                            col = PAD + ((i + u) * Wi + (j0 + 2 * q)) * 256
                            b0 = (u * ncp + q) * 128
                            nc.tensor.matmul(ps[:], gt[:, b0:b0 + 128],
                                             src_rep[:, col:col + 256],
                                             start=(n == 0), stop=(n == last))
                            n += 1
                    dst_write(i, j0, ps)

            h3rep = big2.tile([128, 256 + 81 * 256], bf16, tag="b2")
            nc.gpsimd.memset(h3rep[64:128, 81 * 256:82 * 256], 0.0)

            def wr3(i, j0, ps):
                for t in range(2):
                    j = j0 + t
                    if j >= 9:
                        break
                    p = 9 * i + j
                    d0, d1 = PAD + 256 * p, PAD + 256 * (p + 1)
                    nc.scalar.activation(h3rep[0:64, d0:d1],
                                         ps[64 * t:64 * (t + 1), :], Tanh,
                                         bias=lb3t[:, p:p + 1])
                    nc.vector.tensor_copy(h3rep[64:128, 256 * p:256 * (p + 1)],
                                          h3rep[0:64, d0:d1])

            # ---- lconv3: 13x13 -> 9x9, 5x5 taps ----
            lconv(9, 9, 5, 3, 13, lw3n_d, h2brep, wr3, pre=pre3)

            # h2brep is dead now; h4rep reuses its slot
            h4rep = big1.tile([128, 256 + 25 * 256], bf16, tag="b1")
            nc.gpsimd.memset(h4rep[64:128, 25 * 256:26 * 256], 0.0)

            def wr4(i, j0, ps):
                for t in range(2):
                    j = j0 + t
                    if j >= 5:
                        break
                    p = 5 * i + j
                    d0, d1 = PAD + 256 * p, PAD + 256 * (p + 1)
                    nc.scalar.activation(h4rep[0:64, d0:d1],
                                         ps[64 * t:64 * (t + 1), :], Tanh,
                                         bias=lb4t[:, p:p + 1])
                    nc.vector.tensor_copy(h4rep[64:128, 256 * p:256 * (p + 1)],
                                          h4rep[0:64, d0:d1])

            # ---- lconv4: 9x9 -> 5x5, 5x5 taps ----
            lconv(5, 5, 5, 3, 9, lw4n_d, h3rep, wr4)

            # h3rep is dead after lconv4; h5t reuses big2's slot
            h5t = big2.tile([64, 9 * 256], bf16, tag="b2")

            def wr5(i, j0, ps):
                for t in range(2):
                    j = j0 + t
                    if j >= 3:
                        break
                    p = 3 * i + j
                    nc.scalar.activation(h5t[:, 256 * p:256 * (p + 1)],
                                         ps[64 * t:64 * (t + 1), :], Tanh,
                                         bias=lb5t[:, p:p + 1])

            # ---- lconv5: 5x5 -> 3x3, 3x3 taps ----
            lconv(3, 3, 3, 2, 5, lw5n_d, h4rep, wr5)

            # ---- head: logits[o, b] = sum_f hw[o, f] feat[f, b] ----
            psh = hps.tile([2, 256], f32)
            for yx in range(9):
                nc.tensor.matmul(psh[:], hwch[:, 2 * yx:2 * yx + 2],
                                 h5t[:, 256 * yx:256 * (yx + 1)],
                                 start=(yx == 0), stop=(yx == 8))
            lg = wp.tile([2, 256], f32)
            nc.vector.tensor_copy(lg[:], psh[:])
            nc.sync.dma_start(logits_d[:], lg[:])

    nc.compile()
    return nc


def _prep_weights(w1, b1, w2a, b2a, w2b, b2b, lw3, lb3, lw4, lb4, lw5, lb5, hw):
    """Host-side reshape of weights into the on-chip matmul layouts."""
    out = {}
    # conv1: rows 20*di + 5*g + ci ; cols dj*128 + 32*g + co
    w1di = np.zeros((60, 3, 128), np.float32)
    for dj in range(3):
        for di in range(3):
            blk = w1[:, :, di, dj].T          # [5ci, 32co]
            for g in range(4):
                w1di[20 * di + 5 * g:20 * di + 5 * g + 5, dj,
                     32 * g:32 * g + 32] = blk
    out["w1di"] = w1di.reshape(60, 3 * 128).astype(BF)
    out["b1t"] = np.tile(b1, 4)[:, None].astype(np.float32)

    # conv2a pairs: rows (32*g2 + ci) -> dj=0, rows (64 + 32*g2 + ci) -> dj=1
    w2ap = np.zeros((128, 3, 128), np.float32)
    w2as = np.zeros((64, 3, 128), np.float32)
    for di in range(3):
        b0 = w2a[:, :, di, 0].T               # [32ci, 64co]
        b1_ = w2a[:, :, di, 1].T
        b2_ = w2a[:, :, di, 2].T
        for g2 in range(2):
            w2ap[32 * g2:32 * (g2 + 1), di, 64 * g2:64 * (g2 + 1)] = b0
            w2ap[64 + 32 * g2:64 + 32 * (g2 + 1), di,
                 64 * g2:64 * (g2 + 1)] = b1_
            w2as[32 * g2:32 * (g2 + 1), di, 64 * g2:64 * (g2 + 1)] = b2_
    out["w2ap"] = w2ap.reshape(128, 3 * 128).astype(BF)
    out["w2as"] = w2as.reshape(64, 3 * 128).astype(BF)
    out["b2at"] = np.tile(b2a, 2)[:, None].astype(np.float32)

    TAPS3 = [(di, dj) for di in range(3) for dj in range(3)]
    w2bbd = np.zeros((128, 9, 128), np.float32)
    for t, (di, dj) in enumerate(TAPS3):
        blk = w2b[:, :, di, dj].T             # [64ci, 64co]
        for g2 in range(2):
            w2bbd[64 * g2:64 * (g2 + 1), t, 64 * g2:64 * (g2 + 1)] = blk
    out["w2bbd"] = w2bbd.reshape(128, 9 * 128).astype(BF)
    out["b2bt"] = np.tile(b2b, 2)[:, None].astype(np.float32)

    def lc_pack2(lw, Ho, Wo, kh, kw):
        # per (i, j0) group: kh u-rows x ncp col-pair stationaries [128, 128]
        # K row 64v+ci = input col c+v (v=1 from replica rows); M col
        # 64t+co = output position (i, j0+t).
        ncp = (kw + 1) // 2
        j0s = list(range(0, Wo, 2))
        out_w = np.zeros((Ho * len(j0s), 128, kh * ncp * 128), np.float32)
        g = 0
        for i in range(Ho):
            for j0 in j0s:
                for u in range(kh):
                    for q in range(ncp):
                        c = j0 + 2 * q
                        b0 = (u * ncp + q) * 128
                        for v in range(2):
                            for t in range(2):
                                j = j0 + t
                                if j >= Wo:
                                    continue
                                tap = c + v - j
                                if 0 <= tap < kw:
                                    out_w[g, 64 * v:64 * v + 64,
                                          b0 + 64 * t:b0 + 64 * t + 64] \
                                        = lw[i, j, :, :, u, tap].T
                g += 1
        return out_w.astype(BF)

    out["lw3n"] = lc_pack2(lw3, 9, 9, 5, 5)
    out["lb3t"] = np.ascontiguousarray(
        lb3.transpose(2, 0, 1).reshape(64, 81)).astype(np.float32)
    out["lw4n"] = lc_pack2(lw4, 5, 5, 5, 5)
    out["lb4t"] = np.ascontiguousarray(
        lb4.transpose(2, 0, 1).reshape(64, 25)).astype(np.float32)
    out["lw5n"] = lc_pack2(lw5, 3, 3, 3, 3)
    out["lb5t"] = np.ascontiguousarray(
        lb5.transpose(2, 0, 1).reshape(64, 9)).astype(np.float32)

    out["hwch"] = np.ascontiguousarray(
        hw[:, :576].reshape(2, 64, 9).transpose(1, 2, 0).reshape(64, 18)
    ).astype(BF)
    return out


def kernel(x, info, w1, b1, w2a, b2a, w2b, b2b, lw3, lb3, lw4, lb4, lw5, lb5,
           hw, hb, _trace=False):
    x = np.asarray(x, np.float32)
    if "nc" not in _CACHE:
        _CACHE["nc"] = _build_module()
    nc = _CACHE["nc"]

    wts = _prep_weights(np.asarray(w1, np.float32), np.asarray(b1, np.float32),
                        np.asarray(w2a, np.float32), np.asarray(b2a, np.float32),
                        np.asarray(w2b, np.float32), np.asarray(b2b, np.float32),
                        np.asarray(lw3, np.float32), np.asarray(lb3, np.float32),
                        np.asarray(lw4, np.float32), np.asarray(lb4, np.float32),
                        np.asarray(lw5, np.float32), np.asarray(lb5, np.float32),
                        np.asarray(hw, np.float32))

    xb = np.ascontiguousarray(x.reshape(B_FULL, 5, 3600)).astype(BF)
    in_maps = []
    for c in range(N_CORES):
        m = dict(wts)
        m["x"] = xb[c * B_CORE:(c + 1) * B_CORE]
        in_maps.append(m)

    res = bass_utils.run_bass_kernel_spmd(
        nc, in_maps, core_ids=list(range(N_CORES)), trace=_trace)
    if _trace:
        _CACHE["last_results"] = res

    logits = np.concatenate(
        [res.results[c]["logits"].T for c in range(N_CORES)], axis=0)  # [2048, 2]

    # host-side tail: info contribution + bias + softmax (fp32)
    info = np.asarray(info, np.float32)
    hw = np.asarray(hw, np.float32)
    hb = np.asarray(hb, np.float32)
    logits = logits + info @ hw[:, 576:].T + hb[None, :]
    m = logits.max(axis=1, keepdims=True)
    e = np.exp(logits - m)
    return (e / e.sum(axis=1, keepdims=True)).astype(np.float32)



# revision 36
# speedup vs baseline: 1.2870x; 1.0175x over previous
"""Trainium2 Bass kernel for the DeepFace-style CNN (nn_DeepFace_10574209482846).

Sharding: pure data parallel - batch 2048 split as 256 images per core
across 8 cores; all weights replicated (host-preprocessed into matmul-
friendly layouts, cast to bf16).

v3: software-pipelined phase 1 (conv1 of sb+1 ahead of conv2a of sb,
so the PE never drains on conv1's activations and the HAM clock stays
warm) + conv2b tanh into contiguous staging with the (y,x,b) scatter
moved to the Vector engine (writes base + replica rows directly).

v2: tap-packed K layouts to cut PE stream cycles:
 - conv1: (grp, ci, di) packed on K (K=60), 3 accumulating matmuls (dj)
   over di-shifted x replicas loaded straight from DRAM.
 - conv2a: 2-group pairs with a +1-column replica on partitions 64..127
   (K=128): 3 pair-matmuls (dj=0,1) + 3 singles (dj=2, K=64).
 - conv2b: 9 taps, K=128 2-group block-diagonal (unchanged).
 - locally-connected stack: batch-contiguous (y, x, b) layout with
   +1-position replica rows, tap-pair matmuls (unchanged).
All inter-layer pools double-buffered so sub-batches pipeline with no
PE gaps (keeps the PE HAM clock at 2.4 GHz).
"""

import numpy as np
import concourse.bass as bass
import concourse.bacc as bacc
import concourse.tile as tile
import concourse.mybir as mybir
from concourse import bass_utils

bf16 = mybir.dt.bfloat16
f32 = mybir.dt.float32
BF = mybir.dt.np(bf16)

N_CORES = 8
B_FULL = 2048
B_CORE = 256          # images per core
SB = 8                # images per sub-batch (2 per group)
NSB = B_CORE // SB    # 32
BSB = SB // 4         # 2 images per group per sub-batch

L1 = BSB * 841        # h1 elements per pair tile (flat (b, 29, 29))

_CACHE = {}


def _build_module(nsb=NSB):
    nc = bacc.Bacc("TRN2", target_bir_lowering=False, debug=False,
                   enable_asserts=True, num_devices=N_CORES)

    # ---- DRAM I/O ----
    x_d = nc.dram_tensor("x", [B_CORE, 5, 3600], bf16, kind="ExternalInput").ap()
    w1di_d = nc.dram_tensor("w1di", [60, 3 * 128], bf16, kind="ExternalInput").ap()
    b1t_d = nc.dram_tensor("b1t", [128, 1], f32, kind="ExternalInput").ap()
    w2ap_d = nc.dram_tensor("w2ap", [128, 3 * 128], bf16, kind="ExternalInput").ap()
    w2as_d = nc.dram_tensor("w2as", [64, 3 * 128], bf16, kind="ExternalInput").ap()
    b2at_d = nc.dram_tensor("b2at", [128, 1], f32, kind="ExternalInput").ap()
    w2bbd_d = nc.dram_tensor("w2bbd", [128, 9 * 128], bf16, kind="ExternalInput").ap()
    b2bt_d = nc.dram_tensor("b2bt", [128, 1], f32, kind="ExternalInput").ap()
    lw3n_d = nc.dram_tensor("lw3n", [45, 128, 1920], bf16, kind="ExternalInput").ap()
    lb3_d = nc.dram_tensor("lb3t", [64, 81], f32, kind="ExternalInput").ap()
    lw4n_d = nc.dram_tensor("lw4n", [15, 128, 1920], bf16, kind="ExternalInput").ap()
    lb4_d = nc.dram_tensor("lb4t", [64, 25], f32, kind="ExternalInput").ap()
    lw5n_d = nc.dram_tensor("lw5n", [6, 128, 768], bf16, kind="ExternalInput").ap()
    lb5_d = nc.dram_tensor("lb5t", [64, 9], f32, kind="ExternalInput").ap()
    hwch_d = nc.dram_tensor("hwch", [64, 18], bf16, kind="ExternalInput").ap()
    logits_d = nc.dram_tensor("logits", [2, B_CORE], f32, kind="ExternalOutput").ap()

    Tanh = mybir.ActivationFunctionType.Tanh

    with tile.TileContext(nc) as tc:
        with (
            tc.tile_pool(name="wp", bufs=1) as wp,
            tc.tile_pool(name="lwp", bufs=5) as lwp,
            tc.tile_pool(name="xp", bufs=2) as xp,
            tc.tile_pool(name="h1p", bufs=2) as h1p,
            tc.tile_pool(name="h2ap", bufs=2) as h2ap,
            tc.tile_pool(name="stp", bufs=4) as stp,
            tc.tile_pool(name="big1", bufs=1) as big1,
            tc.tile_pool(name="big2", bufs=1) as big2,
            tc.tile_pool(name="cps", bufs=5, space="PSUM") as cps,
            tc.tile_pool(name="lps", bufs=2, space="PSUM") as lps,
            tc.tile_pool(name="hps", bufs=1, space="PSUM") as hps,
        ):
            # ---- persistent weights (conv1's first, so the first x tile
            # and w1di head the DMA queues and conv1 starts immediately) ----
            w1di = wp.tile([60, 3 * 128], bf16)
            nc.sync.dma_start(w1di[:], w1di_d[:])
            b1t = wp.tile([128, 1], f32)
            nc.sync.dma_start(b1t[:], b1t_d[:])

            def x_dma(sb):
                # x tile: rows 20*di + 5*g + ci hold x[ci] shifted di rows up
                x3 = xp.tile([60, BSB * 3600], bf16, tag="x")
                for di in range(3):
                    for g in range(4):
                        b0 = 64 * g + BSB * sb
                        src = x_d[b0:b0 + BSB, :, 60 * di:].rearrange(
                            "b c m -> c b m")
                        dst = x3[20 * di + 5 * g:20 * di + 5 * g + 5,
                                 :].rearrange(
                            "c (b m) -> c b m", b=BSB)[:, :, :3600 - 60 * di]
                        nc.sync.dma_start(dst, src)
                return x3

            xq = [x_dma(0), x_dma(1)]

            w2ap = wp.tile([128, 3 * 128], bf16)
            nc.sync.dma_start(w2ap[:], w2ap_d[:])
            w2as = wp.tile([64, 3 * 128], bf16)
            nc.sync.dma_start(w2as[:], w2as_d[:])
            b2at = wp.tile([128, 1], f32)
            nc.sync.dma_start(b2at[:], b2at_d[:])
            w2bbd = wp.tile([128, 9 * 128], bf16)
            nc.sync.dma_start(w2bbd[:], w2bbd_d[:])
            b2bt = wp.tile([128, 1], f32)
            nc.sync.dma_start(b2bt[:], b2bt_d[:])
            lb3t = wp.tile([64, 81], f32)
            nc.sync.dma_start(lb3t[:], lb3_d[:])
            lb4t = wp.tile([64, 25], f32)
            nc.sync.dma_start(lb4t[:], lb4_d[:])
            lb5t = wp.tile([64, 9], f32)
            nc.sync.dma_start(lb5t[:], lb5_d[:])
            hwch = wp.tile([64, 18], bf16)
            nc.sync.dma_start(hwch[:], hwch_d[:])

            # ---- persistent activations, batch-contiguous (y, x, b) with a
            # 256-col leading pad; rows 64-127 hold the +1-position replica
            # (element e lives at col PAD+e on rows 0-63 and col e on 64-127).
            # h4rep/h5t reuse the big slots once h2brep/h3rep are dead.
            h2brep = big1.tile([128, 256 + 169 * 256], bf16, tag="b1")
            # zero the one-past-the-end replica column block (read with zero
            # weights by the edge position-pair matmuls; must not be NaN)
            nc.gpsimd.memset(h2brep[64:128, 169 * 256:170 * 256], 0.0)

            # (c, b, p) views of base and replica halves, p = y*13 + x
            h2b_bv = h2brep[0:64, 256:].rearrange("c (p b) -> c b p",
                                                  p=169, b=256)
            h2b_rv = h2brep[64:128, 0:169 * 256].rearrange("c (p b) -> c b p",
                                                           p=169, b=256)

            # ======== phase 1: conv1 -> conv2a -> conv2b, sw-pipelined =====
            # conv1 of sub-batch sb+1 issues before conv2a of sb so the PE
            # never waits on conv1's activation drain (and the HAM clock
            # stays warm); conv2b tanh lands in a contiguous staging tile,
            # the (y,x,b) scatter runs on the idle Vector engine (both base
            # and +1-position replica rows, so no bulk replica copy later).
            def conv1(x3):
                # K=60 (4 groups x 5ci x 3di), 3 dj-matmuls
                xv = x3[:].rearrange("c (b h w) -> c b h w", b=BSB, h=60, w=60)
                h1pr = {}
                for r in range(2):
                    h1pr[r] = h1p.tile([128, 1 + L1 + 3], bf16, tag=f"h1{r}",
                                       name=f"h1pair{r}")
                for (y0, ny) in [(0, 8), (8, 8), (16, 8), (24, 5)]:
                    ps = cps.tile([128, BSB * 8 * 29], f32, tag="cps")
                    psw = ps[:, :BSB * ny * 29]
                    for dj in range(3):
                        rhs = xv[:, :, 2 * y0: 2 * y0 + 2 * ny - 1: 2,
                                 dj: dj + 57: 2]
                        nc.tensor.matmul(psw, w1di[:, 128 * dj:128 * (dj + 1)],
                                         rhs, start=(dj == 0), stop=(dj == 2))
                    psv = psw.rearrange("c (b y x) -> c b y x",
                                        b=BSB, y=ny, x=29)
                    for r in range(2):
                        dstv = h1pr[r][0:64, 1:1 + L1].rearrange(
                            "c (b h w) -> c b h w", b=BSB, h=29, w=29)
                        nc.scalar.activation(dstv[:, :, y0:y0 + ny, :],
                                             psv[64 * r:64 * (r + 1)],
                                             Tanh, bias=b1t[0:64])
                # +1-element replica rows (gives dj+1 taps in pair matmuls)
                for r in range(2):
                    nc.vector.tensor_copy(h1pr[r][64:128, 0:L1],
                                          h1pr[r][0:64, 1:1 + L1])
                return h1pr

            def conv2a(h1pr):
                # per pair, 3 pair-matmuls (K=128) + 3 singles
                h2a_t = {}
                for r in range(2):
                    h2a_t[r] = h2ap.tile([128, BSB * 729], bf16, tag=f"h2a{r}",
                                         name=f"h2a{r}")
                    h2av = h2a_t[r][:].rearrange("c (b h w) -> c b h w",
                                                 b=BSB, h=27, w=27)
                    basev = h1pr[r][:, 1:1 + L1].rearrange(
                        "c (b h w) -> c b h w", b=BSB, h=29, w=29)
                    sglv = h1pr[r][0:64, 3:3 + L1].rearrange(
                        "c (b h w) -> c b h w", b=BSB, h=29, w=29)
                    for (y0, ny) in [(0, 9), (9, 9), (18, 9)]:
                        ps = cps.tile([128, BSB * 9 * 27], f32, tag="cps")
                        psw = ps[:, :BSB * ny * 27]
                        for di in range(3):
                            rhs = basev[:, :, y0 + di: y0 + di + ny, 0:27]
                            nc.tensor.matmul(
                                psw, w2ap[:, 128 * di:128 * (di + 1)], rhs,
                                start=(di == 0), stop=False)
                        for di in range(3):
                            rhs = sglv[:, :, y0 + di: y0 + di + ny, 0:27]
                            nc.tensor.matmul(
                                psw, w2as[:, 128 * di:128 * (di + 1)], rhs,
                                start=False, stop=(di == 2))
                        nc.scalar.activation(h2av[:, :, y0:y0 + ny, :], psw,
                                             Tanh, bias=b2at[:])
                return h2a_t

            TAPS3 = [(di, dj) for di in range(3) for dj in range(3)]

            def conv2b(sb, h2a_t):
                # per pair, K=128 block-diag, stride 2
                for r in range(2):
                    h2av = h2a_t[r][:].rearrange("c (b h w) -> c b h w",
                                                 b=BSB, h=27, w=27)
                    ps = cps.tile([128, BSB * 169], f32, tag="cps")
                    for t, (di, dj) in enumerate(TAPS3):
                        rhs = h2av[:, :, di: di + 25: 2, dj: dj + 25: 2]
                        nc.tensor.matmul(ps[:],
                                         w2bbd[:, 128 * t:128 * (t + 1)],
                                         rhs, start=(t == 0), stop=(t == 8))
                    for g2 in range(2):
                        gb = 64 * (2 * r + g2) + BSB * sb
                        stag = stp.tile([64, BSB * 169], bf16, tag="st")
                        nc.scalar.activation(stag[:],
                                             ps[64 * g2:64 * (g2 + 1), :],
                                             Tanh,
                                             bias=b2bt[64 * g2:64 * (g2 + 1)])
                        sv = stag[:].rearrange("c (b p) -> c b p", b=BSB)
                        nc.vector.tensor_copy(h2b_bv[:, gb:gb + BSB, :], sv)
                        nc.vector.tensor_copy(h2b_rv[:, gb:gb + BSB, :], sv)

            # flat cross-layer weight schedule for the locally-connected
            # stack: a single rolling prefetch window spans layer boundaries
            # and its first entries load during phase 1 (queued after the
            # first x tiles, so they don't delay conv1's start)
            wsched = ([(lw3n_d, gi) for gi in range(45)]
                      + [(lw4n_d, gi) for gi in range(15)]
                      + [(lw5n_d, gi) for gi in range(6)])
            wstate = {"next": 0}
            wtq = []

            def wpump(n=1):
                for _ in range(n):
                    k = wstate["next"]
                    if k >= len(wsched):
                        return
                    d, gi = wsched[k]
                    row = d[gi]
                    t = lwp.tile([128, 1920], bf16, tag="lwn")
                    nc.sync.dma_start(t[:, :row.shape[1]], row)
                    wtq.append(t)
                    wstate["next"] += 1

            wpump(4)

            h1_cur = conv1(xq.pop(0))
            for sb in range(nsb):
                if sb + 2 < nsb:
                    xq.append(x_dma(sb + 2))
                h1_next = conv1(xq.pop(0)) if sb + 1 < nsb else None
                h2a_t = conv2a(h1_cur)
                conv2b(sb, h2a_t)
                h1_cur = h1_next

            # ================= phase 2: locally-connected stack =============
            # Each matmul covers 2 output positions (j0, j0+1) x 64co on M
            # and 2 input columns (c, c+1) x 64ci on K (replica rows supply
            # col c+1), so the full 128x128 array streams N=256 images.
            # Replica rows are written per-position right after each ACT.
            PAD = 256

            def lconv(Ho, Wo, kh, ncp, Wi, src_rep, dst_write):
                groups = [(i, j0) for i in range(Ho) for j0 in range(0, Wo, 2)]
                for gi, (i, j0) in enumerate(groups):
                    wpump(1)
                    gt = wtq.pop(0)
                    ps = lps.tile([128, 256], f32, tag="lps")
                    n, last = 0, kh * ncp - 1
                    for u in range(kh):
                        for q in range(ncp):
                            col = PAD + ((i + u) * Wi + (j0 + 2 * q)) * 256
                            b0 = (u * ncp + q) * 128
                            nc.tensor.matmul(ps[:], gt[:, b0:b0 + 128],
                                             src_rep[:, col:col + 256],
                                             start=(n == 0), stop=(n == last))
                            n += 1
                    dst_write(i, j0, ps)

            h3rep = big2.tile([128, 256 + 81 * 256], bf16, tag="b2")
            nc.gpsimd.memset(h3rep[64:128, 81 * 256:82 * 256], 0.0)

            def wr3(i, j0, ps):
                for t in range(2):
                    j = j0 + t
                    if j >= 9:
                        break
                    p = 9 * i + j
                    d0, d1 = PAD + 256 * p, PAD + 256 * (p + 1)
                    nc.scalar.activation(h3rep[0:64, d0:d1],
                                         ps[64 * t:64 * (t + 1), :], Tanh,
                                         bias=lb3t[:, p:p + 1])
                    nc.vector.tensor_copy(h3rep[64:128, 256 * p:256 * (p + 1)],
                                          h3rep[0:64, d0:d1])

            # ---- lconv3: 13x13 -> 9x9, 5x5 taps ----
            lconv(9, 9, 5, 3, 13, h2brep, wr3)

            # h2brep is dead now; h4rep reuses its slot
            h4rep = big1.tile([128, 256 + 25 * 256], bf16, tag="b1")
            nc.gpsimd.memset(h4rep[64:128, 25 * 256:26 * 256], 0.0)

            def wr4(i, j0, ps):
                for t in range(2):
                    j = j0 + t
                    if j >= 5:
                        break
                    p = 5 * i + j
                    d0, d1 = PAD + 256 * p, PAD + 256 * (p + 1)
                    nc.scalar.activation(h4rep[0:64, d0:d1],
                                         ps[64 * t:64 * (t + 1), :], Tanh,
                                         bias=lb4t[:, p:p + 1])
                    nc.vector.tensor_copy(h4rep[64:128, 256 * p:256 * (p + 1)],
                                          h4rep[0:64, d0:d1])

            # ---- lconv4: 9x9 -> 5x5, 5x5 taps ----
            lconv(5, 5, 5, 3, 9, h3rep, wr4)

            # h3rep is dead after lconv4; h5t reuses big2's slot
            h5t = big2.tile([64, 9 * 256], bf16, tag="b2")

            def wr5(i, j0, ps):
                for t in range(2):
                    j = j0 + t
                    if j >= 3:
                        break
                    p = 3 * i + j
                    nc.scalar.activation(h5t[:, 256 * p:256 * (p + 1)],
                                         ps[64 * t:64 * (t + 1), :], Tanh,
                                         bias=lb5t[:, p:p + 1])

            # ---- lconv5: 5x5 -> 3x3, 3x3 taps ----
            lconv(3, 3, 3, 2, 5, h4rep, wr5)

            # ---- head: logits[o, b] = sum_f hw[o, f] feat[f, b] ----
            psh = hps.tile([2, 256], f32)
            for yx in range(9):
                nc.tensor.matmul(psh[:], hwch[:, 2 * yx:2 * yx + 2],
                                 h5t[:, 256 * yx:256 * (yx + 1)],
                                 start=(yx == 0), stop=(yx == 8))
            lg = wp.tile([2, 256], f32)
            nc.vector.tensor_copy(lg[:], psh[:])
            nc.sync.dma_start(logits_d[:], lg[:])

    nc.compile()
    return nc


def _prep_weights(w1, b1, w2a, b2a, w2b, b2b, lw3, lb3, lw4, lb4, lw5, lb5, hw):
    """Host-side reshape of weights into the on-chip matmul layouts."""
    out = {}
    # conv1: rows 20*di + 5*g + ci ; cols dj*128 + 32*g + co
    w1di = np.zeros((60, 3, 128), np.float32)
    for dj in range(3):
        for di in range(3):
            blk = w1[:, :, di, dj].T          # [5ci, 32co]
            for g in range(4):
                w1di[20 * di + 5 * g:20 * di + 5 * g + 5, dj,
                     32 * g:32 * g + 32] = blk
    out["w1di"] = w1di.reshape(60, 3 * 128).astype(BF)
    out["b1t"] = np.tile(b1, 4)[:, None].astype(np.float32)

    # conv2a pairs: rows (32*g2 + ci) -> dj=0, rows (64 + 32*g2 + ci) -> dj=1
    w2ap = np.zeros((128, 3, 128), np.float32)
    w2as = np.zeros((64, 3, 128), np.float32)
    for di in range(3):
        b0 = w2a[:, :, di, 0].T               # [32ci, 64co]
        b1_ = w2a[:, :, di, 1].T
        b2_ = w2a[:, :, di, 2].T
        for g2 in range(2):
            w2ap[32 * g2:32 * (g2 + 1), di, 64 * g2:64 * (g2 + 1)] = b0
            w2ap[64 + 32 * g2:64 + 32 * (g2 + 1), di,
                 64 * g2:64 * (g2 + 1)] = b1_
            w2as[32 * g2:32 * (g2 + 1), di, 64 * g2:64 * (g2 + 1)] = b2_
    out["w2ap"] = w2ap.reshape(128, 3 * 128).astype(BF)
    out["w2as"] = w2as.reshape(64, 3 * 128).astype(BF)
    out["b2at"] = np.tile(b2a, 2)[:, None].astype(np.float32)

    TAPS3 = [(di, dj) for di in range(3) for dj in range(3)]
    w2bbd = np.zeros((128, 9, 128), np.float32)
    for t, (di, dj) in enumerate(TAPS3):
        blk = w2b[:, :, di, dj].T             # [64ci, 64co]
        for g2 in range(2):
            w2bbd[64 * g2:64 * (g2 + 1), t, 64 * g2:64 * (g2 + 1)] = blk
    out["w2bbd"] = w2bbd.reshape(128, 9 * 128).astype(BF)
    out["b2bt"] = np.tile(b2b, 2)[:, None].astype(np.float32)

    def lc_pack2(lw, Ho, Wo, kh, kw):
        # per (i, j0) group: kh u-rows x ncp col-pair stationaries [128, 128]
        # K row 64v+ci = input col c+v (v=1 from replica rows); M col
        # 64t+co = output position (i, j0+t).
        ncp = (kw + 1) // 2
        j0s = list(range(0, Wo, 2))
        out_w = np.zeros((Ho * len(j0s), 128, kh * ncp * 128), np.float32)
        g = 0
        for i in range(Ho):
            for j0 in j0s:
                for u in range(kh):
                    for q in range(ncp):
                        c = j0 + 2 * q
                        b0 = (u * ncp + q) * 128
                        for v in range(2):
                            for t in range(2):
                                j = j0 + t
                                if j >= Wo:
                                    continue
                                tap = c + v - j
                                if 0 <= tap < kw:
                                    out_w[g, 64 * v:64 * v + 64,
                                          b0 + 64 * t:b0 + 64 * t + 64] \
                                        = lw[i, j, :, :, u, tap].T
                g += 1
        return out_w.astype(BF)

    out["lw3n"] = lc_pack2(lw3, 9, 9, 5, 5)
    out["lb3t"] = np.ascontiguousarray(
        lb3.transpose(2, 0, 1).reshape(64, 81)).astype(np.float32)
    out["lw4n"] = lc_pack2(lw4, 5, 5, 5, 5)
    out["lb4t"] = np.ascontiguousarray(
        lb4.transpose(2, 0, 1).reshape(64, 25)).astype(np.float32)
    out["lw5n"] = lc_pack2(lw5, 3, 3, 3, 3)
    out["lb5t"] = np.ascontiguousarray(
        lb5.transpose(2, 0, 1).reshape(64, 9)).astype(np.float32)

    out["hwch"] = np.ascontiguousarray(
        hw[:, :576].reshape(2, 64, 9).transpose(1, 2, 0).reshape(64, 18)
    ).astype(BF)
    return out


def kernel(x, info, w1, b1, w2a, b2a, w2b, b2b, lw3, lb3, lw4, lb4, lw5, lb5,
           hw, hb, _trace=False):
    x = np.asarray(x, np.float32)
    if "nc" not in _CACHE:
        _CACHE["nc"] = _build_module()
    nc = _CACHE["nc"]

    wts = _prep_weights(np.asarray(w1, np.float32), np.asarray(b1, np.float32),
                        np.asarray(w2a, np.float32), np.asarray(b2a, np.float32),
                        np.asarray(w2b, np.float32), np.asarray(b2b, np.float32),
                        np.asarray(lw3, np.float32), np.asarray(lb3, np.float32),
                        np.asarray(lw4, np.float32), np.asarray(lb4, np.float32),
                        np.asarray(lw5, np.float32), np.asarray(lb5, np.float32),
                        np.asarray(hw, np.float32))

    xb = np.ascontiguousarray(x.reshape(B_FULL, 5, 3600)).astype(BF)
    in_maps = []
    for c in range(N_CORES):
        m = dict(wts)
        m["x"] = xb[c * B_CORE:(c + 1) * B_CORE]
        in_maps.append(m)

    res = bass_utils.run_bass_kernel_spmd(
        nc, in_maps, core_ids=list(range(N_CORES)), trace=_trace)
    if _trace:
        _CACHE["last_results"] = res

    logits = np.concatenate(
        [res.results[c]["logits"].T for c in range(N_CORES)], axis=0)  # [2048, 2]

    # host-side tail: info contribution + bias + softmax (fp32)
    info = np.asarray(info, np.float32)
    hw = np.asarray(hw, np.float32)
    hb = np.asarray(hb, np.float32)
    logits = logits + info @ hw[:, 576:].T + hb[None, :]
    m = logits.max(axis=1, keepdims=True)
    e = np.exp(logits - m)
    return (e / e.sum(axis=1, keepdims=True)).astype(np.float32)

